# revision 26
# baseline (speedup 1.0000x reference)
"""Self-contained GAT message-passing kernel for 8 TRN2 NeuronCores.

kernel(**inputs) takes the full unsharded inputs and returns the full
[100000, 3] output. Nodes are dst-sharded 8 ways. The dense embedder runs
in bf16; per-node 12-value payloads (hh6, a_src2, a_dst2) are packed 4
nodes per 256B table row (32 bf16 lanes each, 12 used) and AllGathered.
The edge phase gathers one 256B quad-row per edge with SWDGE dma_gather
round-robined over 4 SWDGE queues (desc-gen parallelizes ~3x across the
Q7 cores; this is the critical path at ~2.2ns/descriptor) into a
degree-sorted dst-rank grid (idx = gid//4 <= 25002 fits int16, ~1.07x
slot padding), selects the right node sub-row with host-shipped one-hot
masks pre-expanded to the 12 payload lanes (fully packed bf16 multiply +
contiguous tree-adds instead of a strided reduce), and runs the
softmax-weighted reduction per dst block. Self-loops are folded in from
the per-rank DS gather, not grid slots.
"""
import os
import sys

import numpy as np

sys.path.insert(0, "/opt/trn_rl_repo")

from concourse import bass, bacc, mybir, tile, bass_utils
from concourse.masks import make_identity
from dataclasses import dataclass

# walrus must be told to enable dynamic-offset DGE lowering
import concourse.bass_utils as _bu
_orig_run_command = _bu.run_command


def _run_command_dge(cmd, cwd=None):
    if "walrus_driver" in cmd[0]:
        cmd = cmd + [
            "--dge-levels=io,spill_reload,scalar_dynamic_offset,"
            "vector_dynamic_offsets,dynamic_size,dst_reduce"
        ]
        scratch = os.environ.get("GAT_SCRATCH")
        if scratch:
            cmd = [c for c in cmd
                   if not c.startswith("--dynamic-dma-scratch-size-per-partition")]
            cmd = cmd + [f"--dynamic-dma-scratch-size-per-partition={scratch}"]
    return _orig_run_command(cmd, cwd=cwd)


_bu.run_command = _run_command_dge

FP = mybir.dt.float32
BF = mybir.dt.bfloat16
I16 = mybir.dt.int16
AX = mybir.AxisListType
ALU = mybir.AluOpType
ACTF = mybir.ActivationFunctionType

NEG_SLOPE = 0.2
DUMMY_ASRC = -1e38
PACK = 4          # nodes per 256B table row
LANES = 32        # bf16 lanes per node (12 used; 64B record keeps idx in int16)
ROWE = PACK * LANES   # 128 bf16 elements per table row


@dataclass
class Params:
    N: int
    NC: int
    NIN: int
    EH: int
    Dt: tuple = ()        # [NB] slot columns per block
    @property
    def NS(self):
        return self.N // self.NC
    @property
    def NB(self):
        return (self.NS + 127) // 128
    @property
    def NROWS(self):
        return self.NC * (self.NS + 1)
    @property
    def NQ(self):
        assert self.NROWS % PACK == 0
        return self.NROWS // PACK
    @property
    def Sq(self):
        return int(np.sum(self.Dt))


def build_kernel(tc: tile.TileContext, outs, ins, P: Params):
    nc = tc.nc
    xT = ins["xT"]; W1 = ins["W_e1"]; M12 = ins["M12"]; k12 = ins["k12"]
    b_e1 = ins["b_e1"]; BIAS3 = ins["bias3"]
    out = outs["out"]

    NIN, EH, NB = P.NIN, P.EH, P.NB
    NSP = NB * 128
    NK1 = NIN // 128
    NM1 = EH // 128
    assert NIN % 128 == 0 and EH % 128 == 0

    SUPER = 1024
    supers = []
    off = 0
    while off < NSP:
        supers.append((off, min(SUPER, NSP - off)))
        off += SUPER

    dram = tc.alloc_tile_pool(name="dram", bufs=1, space="DRAM")
    # rank-ordered local table; rows NS..NSP-1 are dummy (read by the DS
    # load below), only rows 0..NS take part in the AllGather.
    cc_in = dram.tile([NB * 128, LANES], BF)
    cc_out = dram.tile([P.NROWS, LANES], BF, addr_space="Shared")
    TE = cc_out[:].rearrange("(q f) l -> q (f l)", f=PACK)   # [NQ, 128] bf16

    cpool = tc.alloc_tile_pool(name="cpool", bufs=1)
    bias3s = cpool.tile([128, 4], FP, name="bias3s")
    nc.sync.dma_start(out=bias3s[:], in_=BIAS3[:])

    # ---------------- dense phase (bf16) ----------------
    with tc.tile_pool(name="w1pool", bufs=1) as w1pool:
        # PE operands must be DVE-produced (single sem-wait slot on PE).
        W1r = w1pool.tile([128, NK1, EH], FP, name="W1r")
        for k in range(NK1):
            nc.sync.dma_start(out=W1r[:, k, :], in_=W1[k * 128:(k + 1) * 128, :])
        W1s = w1pool.tile([128, NK1, EH], BF, name="W1s")
        nc.vector.tensor_copy(out=W1s[:], in_=W1r[:])
        M12r = w1pool.tile([128, NM1, 12], FP, name="M12r")
        for k in range(NM1):
            nc.sync.dma_start(out=M12r[:, k, :], in_=M12[k * 128:(k + 1) * 128, :])
        M12s = w1pool.tile([128, NM1, 12], BF, name="M12s")
        nc.vector.tensor_copy(out=M12s[:], in_=M12r[:])
        k12s = w1pool.tile([12, 1], FP, name="k12s")
        nc.sync.dma_start(out=k12s[:], in_=k12[:])
        b1s = w1pool.tile([128, NM1], FP, name="b1s")
        nc.sync.dma_start(out=b1s[:], in_=b_e1.rearrange("(m p) one -> p (m one)", p=128))
        identr = w1pool.tile([128, 128], FP, name="identr")
        make_identity(nc, identr[:])
        ident = w1pool.tile([128, 128], FP, name="ident")
        nc.vector.tensor_copy(out=ident[:], in_=identr[:])

        with tc.tile_pool(name="xin", bufs=2) as xin_pool, \
             tc.tile_pool(name="hT", bufs=2) as hT_pool, \
             tc.tile_pool(name="ps1", bufs=2, space="PSUM") as ps1_pool, \
             tc.tile_pool(name="ps2", bufs=2, space="PSUM") as ps2_pool, \
             tc.tile_pool(name="pst", bufs=2, space="PSUM") as pst_pool, \
             tc.tile_pool(name="tt", bufs=3) as tt_pool, \
             tc.tile_pool(name="tn", bufs=3) as tn_pool:
            for (soff, slen) in supers:
                xr = xin_pool.tile([128, NK1, slen], BF, tag="xr")
                for k in range(NK1):
                    nc.sync.dma_start(
                        out=xr[:, k, :],
                        in_=xT[k * 128:(k + 1) * 128, soff:soff + slen],
                    )
                xt = xin_pool.tile([128, NK1, slen], BF, tag="xt")
                nc.vector.tensor_copy(out=xt[:], in_=xr[:])
                hT = hT_pool.tile([128, NM1, slen], BF, tag="hT")
                nsub = (slen + 511) // 512
                for m in range(NM1):
                    for r in range(nsub):
                        r0 = r * 512
                        rl = min(512, slen - r0)
                        pt = ps1_pool.tile([128, 512], FP, tag="ps1")
                        for k in range(NK1):
                            nc.tensor.matmul(
                                out=pt[:, :rl],
                                lhsT=W1s[:, k, m * 128:(m + 1) * 128],
                                rhs=xt[:, k, r0:r0 + rl],
                                start=(k == 0), stop=(k == NK1 - 1),
                            )
                        nc.vector.tensor_scalar(
                            out=hT[:, m, r0:r0 + rl], in0=pt[:, :rl],
                            scalar1=b1s[:, m:m + 1], scalar2=0.0,
                            op0=ALU.add, op1=ALU.max,
                        )
                for r in range(nsub):
                    r0 = r * 512
                    rl = min(512, slen - r0)
                    pt2 = ps2_pool.tile([12, 512], FP, tag="ps2")
                    for k2 in range(NM1):
                        nc.tensor.matmul(
                            out=pt2[:, :rl],
                            lhsT=M12s[:, k2, :],
                            rhs=hT[:, k2, r0:r0 + rl],
                            start=(k2 == 0), stop=(k2 == NM1 - 1),
                        )
                    tt = tt_pool.tile([12, 512], FP, tag="tt")
                    nc.vector.tensor_scalar(
                        out=tt[:, :rl], in0=pt2[:, :rl],
                        scalar1=k12s[:, :], scalar2=None, op0=ALU.add,
                    )
                    for s in range(0, rl, 128):
                        sl = min(128, rl - s)
                        row0 = soff + r0 + s
                        if row0 >= P.NS:
                            continue
                        e = min(sl, P.NS - row0)
                        ptt = pst_pool.tile([128, 12], FP, tag="pst")
                        nc.tensor.transpose(
                            out=ptt[:sl, :], in_=tt[:, s:s + sl],
                            identity=ident[:12, :12],
                        )
                        tn = tn_pool.tile([128, LANES], BF, tag="tn")
                        nc.vector.tensor_copy(out=tn[:e, 0:12], in_=ptt[:e, :])
                        nc.vector.memset(tn[:e, 12:LANES], 0.0)

                        nc.sync.dma_start(
                            out=cc_in[row0:row0 + e, :], in_=tn[:e, :])

    # dummy rows at local ranks NS..NSP-1: hh=0, a_src=-1e38, a_dst=0
    npad = NSP - P.NS
    dummy = cpool.tile([npad, LANES], BF, name="dummy")
    nc.vector.memset(dummy[:, :], 0.0)
    nc.vector.memset(dummy[:, 6:8], DUMMY_ASRC)
    nc.sync.dma_start(out=cc_in[P.NS:NSP, :], in_=dummy[:])

    # ---------------- allgather ----------------
    nc.gpsimd.collective_compute(
        "AllGather", ALU.bypass,
        replica_groups=[list(range(P.NC))],
        ins=[cc_in[0:P.NS + 1, :].opt()],
        outs=[cc_out[:].opt()],
    )

    # ---------------- edge phase ----------------
    SG = ins["SG"]; OM = ins["OM"]
    Dt = P.Dt
    CHG = max(int(max(Dt)), NB)
    coff = np.zeros(NB, np.int64)
    coff[1:] = np.cumsum(Dt)[:-1]

    with tc.tile_pool(name="ds", bufs=1) as ds_pool, \
         tc.tile_pool(name="acc", bufs=1) as acc_pool, \
         tc.tile_pool(name="sgi", bufs=3) as sg_pool, \
         tc.tile_pool(name="omp", bufs=3) as om_pool, \
         tc.tile_pool(name="gat", bufs=2) as gat_pool, \
         tc.tile_pool(name="tq", bufs=2) as tq_pool, \
         tc.tile_pool(name="sel", bufs=2) as sel_pool, \
         tc.tile_pool(name="ew", bufs=3) as ew_pool:

        # dst-side: the local table is already rank-ordered, so the per-rank
        # own rows are an affine strided read (row b*128+p -> [p, b]) that
        # overlaps the AllGather instead of 13 SWDGE gather calls.
        DSR = ds_pool.tile([128, NB, LANES], BF, name="DSR")
        nc.sync.dma_start(
            out=DSR[:], in_=cc_in[:].rearrange("(b p) l -> p b l", p=128))
        DS = ds_pool.tile([128, NB, 12], FP, name="DS")
        nc.vector.tensor_copy(out=DS[:], in_=DSR[:, :, 0:12])

        # self-loop contribution (per rank): v = a_src + a_dst
        VS = ds_pool.tile([128, NB, 2], FP, name="VS")
        nc.vector.tensor_tensor(
            out=VS[:], in0=DS[:, :, 6:8], in1=DS[:, :, 8:10], op=ALU.add)
        EAS = ds_pool.tile([128, NB, 2], FP, name="EAS")
        nc.scalar.activation(out=EAS[:], in_=VS[:], func=ACTF.Exp, scale=1.0)
        EBS = ds_pool.tile([128, NB, 2], FP, name="EBS")
        nc.scalar.activation(out=EBS[:], in_=VS[:], func=ACTF.Exp, scale=NEG_SLOPE)
        WS = ds_pool.tile([128, NB, 2], FP, name="WS")
        nc.vector.tensor_tensor(out=WS[:], in0=EAS[:], in1=EBS[:], op=ALU.max)

        NUM = acc_pool.tile([128, NB, 2, 3], FP, name="NUM")
        nc.vector.tensor_tensor(
            out=NUM[:],
            in0=WS[:].unsqueeze(-1).to_broadcast([128, NB, 2, 3]),
            in1=DS[:, :, 0:6].rearrange("p b (h c) -> p b h c", h=2),
            op=ALU.mult,
        )
        DEN = acc_pool.tile([128, NB, 2], FP, name="DEN")
        nc.vector.tensor_copy(out=DEN[:], in_=WS[:])

        tc.gat_qn = 0
        for b in range(NB):
            D = int(Dt[b])
            if D == 0:
                continue
            sgs = sg_pool.tile([128, 8 * CHG], I16, tag="sgs")
            nc.sync.dma_start(
                out=sgs[:, 0:8 * D],
                in_=SG[:, 8 * int(coff[b]):8 * int(coff[b] + D)])
            omt = om_pool.tile([128, CHG, PACK, 12], BF, tag="omt")
            nc.sync.dma_start(
                out=omt[:, 0:D, :, :],
                in_=OM[:, PACK * 12 * int(coff[b]):PACK * 12 * int(coff[b] + D)]
                    .rearrange("p (d m l) -> p d m l", m=PACK, l=12))
            G = gat_pool.tile([128, CHG, ROWE], BF, tag="G")
            CH = int(os.environ.get("GAT_CH", "16"))
            for c0 in range(0, D, CH):
                cl = min(CH, D - c0)
                n = 128 * cl
                nc.gpsimd.dma_gather(
                    out_ap=G[:, c0:c0 + cl, :], in_ap=TE,
                    idxs_ap=sgs[:, 8 * c0:8 * (c0 + cl)],
                    num_idxs=n, num_idxs_reg=n, elem_size=ROWE,
                    single_packet=False,
                    queue_num=tc.gat_qn,
                )
                tc.gat_qn = (tc.gat_qn + 1) % 4
            TQ = tq_pool.tile([128, CHG, PACK, 12], BF, tag="TQ")
            nc.vector.tensor_tensor(
                out=TQ[:, 0:D],
                in0=G[:, 0:D].rearrange("p d (m l) -> p d m l", m=PACK)[:, :, :, 0:12],
                in1=omt[:, 0:D],
                op=ALU.mult,
            )
            TH = tq_pool.tile([128, CHG, 2, 12], BF, tag="TH")
            nc.vector.tensor_tensor(
                out=TH[:, 0:D], in0=TQ[:, 0:D, 0:2, :], in1=TQ[:, 0:D, 2:4, :],
                op=ALU.add,
            )
            SEL = sel_pool.tile([128, CHG, 12], FP, tag="SEL")
            nc.vector.tensor_tensor(
                out=SEL[:, 0:D], in0=TH[:, 0:D, 0, :], in1=TH[:, 0:D, 1, :],
                op=ALU.add,
            )
            V = ew_pool.tile([128, CHG, 2], FP, tag="V")
            nc.vector.tensor_tensor(
                out=V[:, 0:D], in0=SEL[:, 0:D, 6:8],
                in1=DS[:, b:b + 1, 8:10].to_broadcast([128, D, 2]),
                op=ALU.add,
            )
            EA = ew_pool.tile([128, CHG, 2], FP, tag="EA")
            nc.scalar.activation(out=EA[:, 0:D], in_=V[:, 0:D], func=ACTF.Exp, scale=1.0)
            EB = ew_pool.tile([128, CHG, 2], FP, tag="EB")
            nc.scalar.activation(out=EB[:, 0:D], in_=V[:, 0:D], func=ACTF.Exp, scale=NEG_SLOPE)
            W = ew_pool.tile([128, CHG, 2], FP, tag="W")
            nc.vector.tensor_tensor(
                out=W[:, 0:D], in0=EA[:, 0:D], in1=EB[:, 0:D], op=ALU.max)
            DENP = ew_pool.tile([128, 2], FP, tag="DENP")
            nc.vector.tensor_reduce(
                out=DENP[:], in_=W[:, 0:D].rearrange("p d h -> p h d"),
                axis=AX.X, op=ALU.add,
            )
            nc.vector.tensor_tensor(
                out=DEN[:, b, :], in0=DEN[:, b, :], in1=DENP[:], op=ALU.add)
            TMP = ew_pool.tile([128, CHG, 2, 3], FP, tag="TMP")
            nc.vector.tensor_tensor(
                out=TMP[:, 0:D],
                in0=W[:, 0:D].unsqueeze(-1).to_broadcast([128, D, 2, 3]),
                in1=SEL[:, 0:D, 0:6].rearrange("p d (h c) -> p d h c", h=2),
                op=ALU.mult,
            )
            NUMP = ew_pool.tile([128, 6], FP, tag="NUMP")
            nc.vector.tensor_reduce(
                out=NUMP[:], in_=TMP[:, 0:D].rearrange("p d h c -> p (h c) d"),
                axis=AX.X, op=ALU.add,
            )
            nc.vector.tensor_tensor(
                out=NUM[:, b], in0=NUM[:, b],
                in1=NUMP[:].rearrange("p (h c) -> p h c", h=2), op=ALU.add)

        # ---------------- final combine ----------------
        REC = acc_pool.tile([128, NB, 2], FP, name="REC")
        nc.vector.reciprocal(out=REC[:], in_=DEN[:])
        T1 = acc_pool.tile([128, NB, 2, 3], FP, name="T1")
        nc.vector.tensor_tensor(
            out=T1[:], in0=NUM[:],
            in1=REC[:].unsqueeze(-1).to_broadcast([128, NB, 2, 3]),
            op=ALU.mult,
        )
        O3 = acc_pool.tile([128, NB, 3], FP, name="O3")
        nc.vector.tensor_tensor(
            out=O3[:], in0=T1[:, :, 0, :], in1=T1[:, :, 1, :], op=ALU.add)
        nc.vector.tensor_scalar_mul(O3[:], O3[:], 0.5)
        nc.vector.tensor_tensor(
            out=O3[:],
            in0=O3[:],
            in1=bias3s[:, 0:3].unsqueeze(1).to_broadcast([128, NB, 3]),
            op=ALU.add,
        )
        nc.sync.dma_start(out=out[:], in_=O3[:].rearrange("p b c -> p (b c)"))
    cpool.release()
    dram.release()


# ====================== host side ======================

def fuse_weights(W_e1, b_e1, W_e2, b_e2, W_lin, b_lin, W_att, att_src, att_dst):
    W64 = lambda a: a.astype(np.float64)
    WL = W64(W_e2) @ W64(W_lin) @ W64(W_att)
    kL = W64(b_e2) @ W64(W_lin) @ W64(W_att) + W64(b_lin) @ W64(W_att)
    A_s = np.zeros((6, 2)); A_d = np.zeros((6, 2))
    for h in range(2):
        for c in range(3):
            A_s[3 * h + c, h] = att_src[h, c]
            A_d[3 * h + c, h] = att_dst[h, c]
    EH = W_e1.shape[1]
    M12 = np.zeros((EH, 12), np.float32)
    M12[:, :6] = WL.astype(np.float32)
    M12[:, 6:8] = (WL @ A_s).astype(np.float32)
    M12[:, 8:10] = (WL @ A_d).astype(np.float32)
    k12 = np.zeros((12, 1), np.float32)
    k12[:6, 0] = kL.astype(np.float32)
    k12[6:8, 0] = (kL @ A_s).astype(np.float32)
    k12[8:10, 0] = (kL @ A_d).astype(np.float32)
    return M12, k12


def wrap_idx16(lst):
    """[n] int -> [128, n//16] int16 wrapped+replicated layout."""
    n = len(lst)
    assert n % 16 == 0
    w = np.asarray(lst, np.int16).reshape(n // 16, 16).T  # [16, n/16]
    return np.tile(w, (8, 1))


def onehot8_bf16(m):
    """[...]-shaped int array of sub-row ids -> [..., 8] bf16-as-uint16 one-hot."""
    import ml_dtypes
    oh = (m[..., None] == np.arange(PACK)).astype(ml_dtypes.bfloat16)
    return oh


def prepare_inputs(inputs, P: Params):
    import ml_dtypes
    x = np.asarray(inputs["x"]); ei = np.asarray(inputs["edge_index"])
    M12, k12 = fuse_weights(
        inputs["W_e1"], inputs["b_e1"], inputs["W_e2"], inputs["b_e2"],
        inputs["W_lin"], inputs["b_lin"], inputs["W_att"],
        inputs["att_src"], inputs["att_dst"])
    src = ei[0].astype(np.int64); dst = ei[1].astype(np.int64)
    NS, NB, NC = P.NS, P.NB, P.NC
    core_of = dst // NS

    # shared degree-sort permutation per core + shared per-block col budget
    perms = []
    rankof_full = np.empty(P.N, np.int64)
    Dt = np.zeros(NB, np.int64)
    for c in range(NC):
        m = core_of == c
        dloc = dst[m] - c * NS
        deg = np.bincount(dloc, minlength=NS)
        order = np.argsort(-deg, kind="stable")
        perms.append(order)
        rank_of = np.empty(NS, np.int64)
        rank_of[order] = np.arange(NS)
        rankof_full[c * NS:(c + 1) * NS] = rank_of
        degp = np.zeros(NB * 128, np.int64)
        degp[:NS] = deg[order]
        Dt = np.maximum(Dt, degp.reshape(NB, 128).max(1))
    P.Dt = tuple(int(v) for v in Dt)
    # table rows are rank-ordered per core: gid of node n = core*(NS+1)+rank(n)
    gid_src = (src // NS) * (NS + 1) + rankof_full[src]
    Sq = P.Sq
    coff = np.zeros(NB, np.int64)
    coff[1:] = np.cumsum(Dt)[:-1]

    b_e1c = inputs["b_e1"].reshape(-1, 1).astype(np.float32)
    bias3 = np.zeros((128, 4), np.float32)
    bias3[:, :3] = inputs["bias"]

    in_maps = []
    for c in range(NC):
        m = core_of == c
        src_c = src[m]; dloc_c = dst[m] - c * NS
        gid_c = gid_src[m]
        perm = perms[c]
        rank_of = np.empty(NS, np.int64)
        rank_of[perm] = np.arange(NS)

        gid_dummy = c * (NS + 1) + NS
        # flat slot arrays (col-major within block: slot = (coff[b]+k)*128+p)
        LQ = np.full(128 * Sq, gid_dummy // PACK, np.int64)
        MQ = np.full(128 * Sq, gid_dummy % PACK, np.int64)

        r = rank_of[dloc_c]
        order_e = np.argsort(r, kind="stable")
        rs = r[order_e]; gs = gid_c[order_e]
        starts = np.searchsorted(rs, np.arange(NB * 128), side="left")
        k = np.arange(len(rs)) - starts[rs]
        p = rs % 128; b = rs // 128
        pos = (coff[b] + k) * 128 + p
        LQ[pos] = gs // PACK
        MQ[pos] = gs % PACK
        assert LQ.max() < P.NQ and LQ.min() >= 0

        # one-hot masks pre-expanded to the 12 payload lanes:
        # [128 p, Sq, PACK, 12] so the select multiply is fully packed bf16
        MQ_grid = MQ.reshape(Sq, 128).T           # [128, Sq]
        oh = onehot8_bf16(MQ_grid)                # [128, Sq, PACK]
        OM = np.ascontiguousarray(
            np.broadcast_to(oh[..., None], (128, Sq, PACK, 12))
        ).reshape(128, Sq * PACK * 12)

        # dense phase processes nodes in rank order (table rows = ranks)
        xs = np.zeros((P.NIN, NB * 128), ml_dtypes.bfloat16)
        xs[:, :NS] = x[c * NS:(c + 1) * NS][perm].T.astype(ml_dtypes.bfloat16)
        in_maps.append({
            "xT": xs, "W_e1": np.asarray(inputs["W_e1"]), "M12": M12, "k12": k12,
            "b_e1": b_e1c, "bias3": bias3,
            "SG": wrap_idx16(LQ), "OM": OM,
        })

    def post(results):
        outf = np.zeros((P.N, 3), np.float32)
        for c in range(NC):
            o = results[c]["out"].reshape(128, NB, 3)
            grid = np.transpose(o, (1, 0, 2)).reshape(NB * 128, 3)
            outf[c * NS + perms[c]] = grid[:NS]
        return outf

    return in_maps, post, perms


# ====================== entry point ======================

_CACHE = {}
last_exec_time_ns = None
last_result = None


def kernel(**inputs) -> np.ndarray:
    global last_exec_time_ns, last_result
    P = Params(N=100000, NC=8, NIN=768, EH=512)
    in_maps, post, _perms = prepare_inputs(inputs, P)

    key = ("gat8v2", P.N, P.Dt)
    if key not in _CACHE:
        nc = bacc.Bacc("TRN2", target_bir_lowering=False, debug=False,
                       num_devices=P.NC, num_swdge_queues=4)
        ins_ap = {}
        for name, arr in in_maps[0].items():
            ins_ap[name] = nc.dram_tensor(
                name, list(arr.shape), mybir.dt.from_np(arr.dtype),
                kind="ExternalInput").ap()
        out_ap = {"out": nc.dram_tensor(
            "out", [128, P.NB * 3], FP, kind="ExternalOutput").ap()}
        with tile.TileContext(nc) as tc:
            build_kernel(tc, out_ap, ins_ap, P)
        nc.compile()
        _CACHE[key] = nc
    nc = _CACHE[key]

    trace = os.environ.get("GAT_TRACE", "0") == "1"
    tmpdir = os.environ.get("GAT_TRACE_DIR") or None
    res = bass_utils.run_bass_kernel_spmd(
        nc, in_maps, core_ids=list(range(P.NC)), trace=trace, tmpdir=tmpdir)
    last_exec_time_ns = res.exec_time_ns
    last_result = res
    return post(res.results)



# revision 28
# speedup vs baseline: 1.0222x; 1.0222x over previous
"""Self-contained GAT message-passing kernel for 8 TRN2 NeuronCores.

kernel(**inputs) takes the full unsharded inputs and returns the full
[100000, 3] output. Nodes are dst-sharded 8 ways. The dense embedder runs
in bf16; per-node 12-value payloads (hh6, a_src2, a_dst2) are packed 4
nodes per 256B table row (32 bf16 lanes each, 12 used) and AllGathered.
The edge phase gathers one 256B quad-row per edge with SWDGE dma_gather
round-robined over 4 SWDGE queues (desc-gen parallelizes ~3x across the
Q7 cores; this is the critical path at ~2.2ns/descriptor) into a
degree-sorted dst-rank grid (idx = gid//4 <= 25002 fits int16, ~1.07x
slot padding), selects the right node sub-row with host-shipped one-hot
masks pre-expanded to the 12 payload lanes (fully packed bf16 multiply +
contiguous tree-adds instead of a strided reduce), and runs the
softmax-weighted reduction per dst block. Self-loops are folded in from
the per-rank DS gather, not grid slots.
"""
import os
import sys

import numpy as np

sys.path.insert(0, "/opt/trn_rl_repo")

from concourse import bass, bacc, mybir, tile, bass_utils
from concourse.masks import make_identity
from dataclasses import dataclass

# walrus must be told to enable dynamic-offset DGE lowering
import concourse.bass_utils as _bu
_orig_run_command = _bu.run_command


def _run_command_dge(cmd, cwd=None):
    if "walrus_driver" in cmd[0]:
        cmd = cmd + [
            "--dge-levels=io,spill_reload,scalar_dynamic_offset,"
            "vector_dynamic_offsets,dynamic_size,dst_reduce"
        ]
        scratch = os.environ.get("GAT_SCRATCH")
        if scratch:
            cmd = [c for c in cmd
                   if not c.startswith("--dynamic-dma-scratch-size-per-partition")]
            cmd = cmd + [f"--dynamic-dma-scratch-size-per-partition={scratch}"]
    return _orig_run_command(cmd, cwd=cwd)


_bu.run_command = _run_command_dge

FP = mybir.dt.float32
BF = mybir.dt.bfloat16
I16 = mybir.dt.int16
AX = mybir.AxisListType
ALU = mybir.AluOpType
ACTF = mybir.ActivationFunctionType

NEG_SLOPE = 0.2
DUMMY_ASRC = -1e38
PACK = 4          # nodes per 256B table row
LANES = 32        # bf16 lanes per node (12 used; 64B record keeps idx in int16)
ROWE = PACK * LANES   # 128 bf16 elements per table row


@dataclass
class Params:
    N: int
    NC: int
    NIN: int
    EH: int
    Dt: tuple = ()        # [NB] slot columns per block
    @property
    def NS(self):
        return self.N // self.NC
    @property
    def NB(self):
        return (self.NS + 127) // 128
    @property
    def NROWS(self):
        return self.NC * (self.NS + 1)
    @property
    def NQ(self):
        assert self.NROWS % PACK == 0
        return self.NROWS // PACK
    @property
    def Sq(self):
        return int(np.sum(self.Dt))


def build_kernel(tc: tile.TileContext, outs, ins, P: Params):
    nc = tc.nc
    xT = ins["xT"]; W1 = ins["W_e1"]; M12 = ins["M12"]; k12 = ins["k12"]
    b_e1 = ins["b_e1"]; BIAS3 = ins["bias3"]
    out = outs["out"]

    NIN, EH, NB = P.NIN, P.EH, P.NB
    NSP = NB * 128
    NK1 = NIN // 128
    NM1 = EH // 128
    assert NIN % 128 == 0 and EH % 128 == 0

    SUPER = 1024
    supers = []
    off = 0
    while off < NSP:
        supers.append((off, min(SUPER, NSP - off)))
        off += SUPER

    dram = tc.alloc_tile_pool(name="dram", bufs=1, space="DRAM")
    # rank-ordered local table; rows NS..NSP-1 are dummy (read by the DS
    # load below), only rows 0..NS take part in the AllGather.
    cc_in = dram.tile([NB * 128, LANES], BF)
    cc_out = dram.tile([P.NROWS, LANES], BF, addr_space="Shared")
    TE = cc_out[:].rearrange("(q f) l -> q (f l)", f=PACK)   # [NQ, 128] bf16

    cpool = tc.alloc_tile_pool(name="cpool", bufs=1)
    bias3s = cpool.tile([128, 4], FP, name="bias3s")
    nc.sync.dma_start(out=bias3s[:], in_=BIAS3[:])

    # ---------------- dense phase (bf16) ----------------
    with tc.tile_pool(name="w1pool", bufs=1) as w1pool:
        # PE operands must be DVE-produced (single sem-wait slot on PE).
        W1r = w1pool.tile([128, NK1, EH], FP, name="W1r")
        for k in range(NK1):
            nc.sync.dma_start(out=W1r[:, k, :], in_=W1[k * 128:(k + 1) * 128, :])
        W1s = w1pool.tile([128, NK1, EH], BF, name="W1s")
        nc.vector.tensor_copy(out=W1s[:], in_=W1r[:])
        M12r = w1pool.tile([128, NM1, 12], FP, name="M12r")
        for k in range(NM1):
            nc.sync.dma_start(out=M12r[:, k, :], in_=M12[k * 128:(k + 1) * 128, :])
        M12s = w1pool.tile([128, NM1, 12], BF, name="M12s")
        nc.vector.tensor_copy(out=M12s[:], in_=M12r[:])
        k12s = w1pool.tile([12, 1], FP, name="k12s")
        nc.sync.dma_start(out=k12s[:], in_=k12[:])
        b1s = w1pool.tile([128, NM1], FP, name="b1s")
        nc.sync.dma_start(out=b1s[:], in_=b_e1.rearrange("(m p) one -> p (m one)", p=128))
        identr = w1pool.tile([128, 128], FP, name="identr")
        make_identity(nc, identr[:])
        ident = w1pool.tile([128, 128], FP, name="ident")
        nc.vector.tensor_copy(out=ident[:], in_=identr[:])

        with tc.tile_pool(name="xin", bufs=2) as xin_pool, \
             tc.tile_pool(name="hT", bufs=2) as hT_pool, \
             tc.tile_pool(name="ps1", bufs=2, space="PSUM") as ps1_pool, \
             tc.tile_pool(name="ps2", bufs=2, space="PSUM") as ps2_pool, \
             tc.tile_pool(name="pst", bufs=2, space="PSUM") as pst_pool, \
             tc.tile_pool(name="tt", bufs=3) as tt_pool, \
             tc.tile_pool(name="tn", bufs=3) as tn_pool:
            for (soff, slen) in supers:
                xr = xin_pool.tile([128, NK1, slen], BF, tag="xr")
                for k in range(NK1):
                    nc.sync.dma_start(
                        out=xr[:, k, :],
                        in_=xT[k * 128:(k + 1) * 128, soff:soff + slen],
                    )
                xt = xin_pool.tile([128, NK1, slen], BF, tag="xt")
                nc.vector.tensor_copy(out=xt[:], in_=xr[:])
                hT = hT_pool.tile([128, NM1, slen], BF, tag="hT")
                CW = 512
                nsub = (slen + CW - 1) // CW
                for m in range(NM1):
                    for r in range(nsub):
                        r0 = r * CW
                        rl = min(CW, slen - r0)
                        pt = ps1_pool.tile([128, CW], FP, tag="ps1")
                        for k in range(NK1):
                            nc.tensor.matmul(
                                out=pt[:, :rl],
                                lhsT=W1s[:, k, m * 128:(m + 1) * 128],
                                rhs=xt[:, k, r0:r0 + rl],
                                start=(k == 0), stop=(k == NK1 - 1),
                            )
                        nc.vector.tensor_scalar(
                            out=hT[:, m, r0:r0 + rl], in0=pt[:, :rl],
                            scalar1=b1s[:, m:m + 1], scalar2=0.0,
                            op0=ALU.add, op1=ALU.max,
                        )
                for r in range(nsub):
                    r0 = r * 512
                    rl = min(512, slen - r0)
                    pt2 = ps2_pool.tile([12, 512], FP, tag="ps2")
                    for k2 in range(NM1):
                        nc.tensor.matmul(
                            out=pt2[:, :rl],
                            lhsT=M12s[:, k2, :],
                            rhs=hT[:, k2, r0:r0 + rl],
                            start=(k2 == 0), stop=(k2 == NM1 - 1),
                        )
                    tt = tt_pool.tile([12, 512], FP, tag="tt")
                    nc.vector.tensor_scalar(
                        out=tt[:, :rl], in0=pt2[:, :rl],
                        scalar1=k12s[:, :], scalar2=None, op0=ALU.add,
                    )
                    for s in range(0, rl, 128):
                        sl = min(128, rl - s)
                        row0 = soff + r0 + s
                        if row0 >= P.NS:
                            continue
                        e = min(sl, P.NS - row0)
                        ptt = pst_pool.tile([128, 12], FP, tag="pst")
                        nc.tensor.transpose(
                            out=ptt[:sl, :], in_=tt[:, s:s + sl],
                            identity=ident[:12, :12],
                        )
                        tn = tn_pool.tile([128, LANES], BF, tag="tn")
                        nc.vector.tensor_copy(out=tn[:e, 0:12], in_=ptt[:e, :])
                        nc.vector.memset(tn[:e, 12:LANES], 0.0)

                        nc.sync.dma_start(
                            out=cc_in[row0:row0 + e, :], in_=tn[:e, :])

    # dummy rows at local ranks NS..NSP-1: hh=0, a_src=-1e38, a_dst=0
    npad = NSP - P.NS
    dummy = cpool.tile([npad, LANES], BF, name="dummy")
    nc.vector.memset(dummy[:, :], 0.0)
    nc.vector.memset(dummy[:, 6:8], DUMMY_ASRC)
    nc.sync.dma_start(out=cc_in[P.NS:NSP, :], in_=dummy[:])

    # ---------------- allgather ----------------
    nc.gpsimd.collective_compute(
        "AllGather", ALU.bypass,
        replica_groups=[list(range(P.NC))],
        ins=[cc_in[0:P.NS + 1, :].opt()],
        outs=[cc_out[:].opt()],
    )

    # ---------------- edge phase ----------------
    SG = ins["SG"]; OM = ins["OM"]
    Dt = P.Dt
    CHG = max(int(max(Dt)), NB)
    coff = np.zeros(NB, np.int64)
    coff[1:] = np.cumsum(Dt)[:-1]

    with tc.tile_pool(name="ds", bufs=1) as ds_pool, \
         tc.tile_pool(name="acc", bufs=1) as acc_pool, \
         tc.tile_pool(name="sgi", bufs=3) as sg_pool, \
         tc.tile_pool(name="omp", bufs=3) as om_pool, \
         tc.tile_pool(name="gat", bufs=2) as gat_pool, \
         tc.tile_pool(name="tq", bufs=2) as tq_pool, \
         tc.tile_pool(name="sel", bufs=2) as sel_pool, \
         tc.tile_pool(name="ew", bufs=3) as ew_pool:

        # dst-side: the local table is already rank-ordered, so the per-rank
        # own rows are an affine strided read (row b*128+p -> [p, b]) that
        # overlaps the AllGather instead of 13 SWDGE gather calls.
        DSR = ds_pool.tile([128, NB, LANES], BF, name="DSR")
        nc.sync.dma_start(
            out=DSR[:], in_=cc_in[:].rearrange("(b p) l -> p b l", p=128))
        DS = ds_pool.tile([128, NB, 12], FP, name="DS")
        nc.vector.tensor_copy(out=DS[:], in_=DSR[:, :, 0:12])

        # self-loop contribution (per rank): v = a_src + a_dst
        VS = ds_pool.tile([128, NB, 2], FP, name="VS")
        nc.vector.tensor_tensor(
            out=VS[:], in0=DS[:, :, 6:8], in1=DS[:, :, 8:10], op=ALU.add)
        EAS = ds_pool.tile([128, NB, 2], FP, name="EAS")
        nc.scalar.activation(out=EAS[:], in_=VS[:], func=ACTF.Exp, scale=1.0)
        EBS = ds_pool.tile([128, NB, 2], FP, name="EBS")
        nc.scalar.activation(out=EBS[:], in_=VS[:], func=ACTF.Exp, scale=NEG_SLOPE)
        WS = ds_pool.tile([128, NB, 2], FP, name="WS")
        nc.vector.tensor_tensor(out=WS[:], in0=EAS[:], in1=EBS[:], op=ALU.max)

        NUM = acc_pool.tile([128, NB, 2, 3], FP, name="NUM")
        nc.vector.tensor_tensor(
            out=NUM[:],
            in0=WS[:].unsqueeze(-1).to_broadcast([128, NB, 2, 3]),
            in1=DS[:, :, 0:6].rearrange("p b (h c) -> p b h c", h=2),
            op=ALU.mult,
        )
        DEN = acc_pool.tile([128, NB, 2], FP, name="DEN")
        nc.vector.tensor_copy(out=DEN[:], in_=WS[:])

        tc.gat_qn = 0
        for b in range(NB):
            D = int(Dt[b])
            if D == 0:
                continue
            sgs = sg_pool.tile([128, 8 * CHG], I16, tag="sgs")
            nc.sync.dma_start(
                out=sgs[:, 0:8 * D],
                in_=SG[:, 8 * int(coff[b]):8 * int(coff[b] + D)])
            omt = om_pool.tile([128, CHG, PACK, 12], BF, tag="omt")
            nc.sync.dma_start(
                out=omt[:, 0:D, :, :],
                in_=OM[:, PACK * 12 * int(coff[b]):PACK * 12 * int(coff[b] + D)]
                    .rearrange("p (d m l) -> p d m l", m=PACK, l=12))
            G = gat_pool.tile([128, CHG, ROWE], BF, tag="G")
            CH = int(os.environ.get("GAT_CH", "16"))
            for c0 in range(0, D, CH):
                cl = min(CH, D - c0)
                n = 128 * cl
                nc.gpsimd.dma_gather(
                    out_ap=G[:, c0:c0 + cl, :], in_ap=TE,
                    idxs_ap=sgs[:, 8 * c0:8 * (c0 + cl)],
                    num_idxs=n, num_idxs_reg=n, elem_size=ROWE,
                    single_packet=False,
                    queue_num=tc.gat_qn,
                )
                tc.gat_qn = (tc.gat_qn + 1) % 4
            TQ = tq_pool.tile([128, CHG, PACK, 12], BF, tag="TQ")
            nc.vector.tensor_tensor(
                out=TQ[:, 0:D],
                in0=G[:, 0:D].rearrange("p d (m l) -> p d m l", m=PACK)[:, :, :, 0:12],
                in1=omt[:, 0:D],
                op=ALU.mult,
            )
            TH = tq_pool.tile([128, CHG, 2, 12], BF, tag="TH")
            nc.vector.tensor_tensor(
                out=TH[:, 0:D], in0=TQ[:, 0:D, 0:2, :], in1=TQ[:, 0:D, 2:4, :],
                op=ALU.add,
            )
            SEL = sel_pool.tile([128, CHG, 12], FP, tag="SEL")
            nc.vector.tensor_tensor(
                out=SEL[:, 0:D], in0=TH[:, 0:D, 0, :], in1=TH[:, 0:D, 1, :],
                op=ALU.add,
            )
            V = ew_pool.tile([128, CHG, 2], FP, tag="V")
            nc.vector.tensor_tensor(
                out=V[:, 0:D], in0=SEL[:, 0:D, 6:8],
                in1=DS[:, b:b + 1, 8:10].to_broadcast([128, D, 2]),
                op=ALU.add,
            )
            EA = ew_pool.tile([128, CHG, 2], FP, tag="EA")
            nc.scalar.activation(out=EA[:, 0:D], in_=V[:, 0:D], func=ACTF.Exp, scale=1.0)
            EB = ew_pool.tile([128, CHG, 2], FP, tag="EB")
            nc.scalar.activation(out=EB[:, 0:D], in_=V[:, 0:D], func=ACTF.Exp, scale=NEG_SLOPE)
            W = ew_pool.tile([128, CHG, 2], FP, tag="W")
            nc.vector.tensor_tensor(
                out=W[:, 0:D], in0=EA[:, 0:D], in1=EB[:, 0:D], op=ALU.max)
            DENP = ew_pool.tile([128, 2], FP, tag="DENP")
            nc.vector.tensor_reduce(
                out=DENP[:], in_=W[:, 0:D].rearrange("p d h -> p h d"),
                axis=AX.X, op=ALU.add,
            )
            nc.vector.tensor_tensor(
                out=DEN[:, b, :], in0=DEN[:, b, :], in1=DENP[:], op=ALU.add)
            TMP = ew_pool.tile([128, CHG, 2, 3], FP, tag="TMP")
            nc.vector.tensor_tensor(
                out=TMP[:, 0:D],
                in0=W[:, 0:D].unsqueeze(-1).to_broadcast([128, D, 2, 3]),
                in1=SEL[:, 0:D, 0:6].rearrange("p d (h c) -> p d h c", h=2),
                op=ALU.mult,
            )
            NUMP = ew_pool.tile([128, 6], FP, tag="NUMP")
            nc.vector.tensor_reduce(
                out=NUMP[:], in_=TMP[:, 0:D].rearrange("p d h c -> p (h c) d"),
                axis=AX.X, op=ALU.add,
            )
            nc.vector.tensor_tensor(
                out=NUM[:, b], in0=NUM[:, b],
                in1=NUMP[:].rearrange("p (h c) -> p h c", h=2), op=ALU.add)

        # ---------------- final combine ----------------
        REC = acc_pool.tile([128, NB, 2], FP, name="REC")
        nc.vector.reciprocal(out=REC[:], in_=DEN[:])
        T1 = acc_pool.tile([128, NB, 2, 3], FP, name="T1")
        nc.vector.tensor_tensor(
            out=T1[:], in0=NUM[:],
            in1=REC[:].unsqueeze(-1).to_broadcast([128, NB, 2, 3]),
            op=ALU.mult,
        )
        O3 = acc_pool.tile([128, NB, 3], FP, name="O3")
        nc.vector.tensor_tensor(
            out=O3[:], in0=T1[:, :, 0, :], in1=T1[:, :, 1, :], op=ALU.add)
        nc.vector.tensor_scalar_mul(O3[:], O3[:], 0.5)
        nc.vector.tensor_tensor(
            out=O3[:],
            in0=O3[:],
            in1=bias3s[:, 0:3].unsqueeze(1).to_broadcast([128, NB, 3]),
            op=ALU.add,
        )
        nc.sync.dma_start(out=out[:], in_=O3[:].rearrange("p b c -> p (b c)"))
    cpool.release()
    dram.release()


# ====================== host side ======================

def fuse_weights(W_e1, b_e1, W_e2, b_e2, W_lin, b_lin, W_att, att_src, att_dst):
    W64 = lambda a: a.astype(np.float64)
    WL = W64(W_e2) @ W64(W_lin) @ W64(W_att)
    kL = W64(b_e2) @ W64(W_lin) @ W64(W_att) + W64(b_lin) @ W64(W_att)
    A_s = np.zeros((6, 2)); A_d = np.zeros((6, 2))
    for h in range(2):
        for c in range(3):
            A_s[3 * h + c, h] = att_src[h, c]
            A_d[3 * h + c, h] = att_dst[h, c]
    EH = W_e1.shape[1]
    M12 = np.zeros((EH, 12), np.float32)
    M12[:, :6] = WL.astype(np.float32)
    M12[:, 6:8] = (WL @ A_s).astype(np.float32)
    M12[:, 8:10] = (WL @ A_d).astype(np.float32)
    k12 = np.zeros((12, 1), np.float32)
    k12[:6, 0] = kL.astype(np.float32)
    k12[6:8, 0] = (kL @ A_s).astype(np.float32)
    k12[8:10, 0] = (kL @ A_d).astype(np.float32)
    return M12, k12


def wrap_idx16(lst):
    """[n] int -> [128, n//16] int16 wrapped+replicated layout."""
    n = len(lst)
    assert n % 16 == 0
    w = np.asarray(lst, np.int16).reshape(n // 16, 16).T  # [16, n/16]
    return np.tile(w, (8, 1))


def onehot8_bf16(m):
    """[...]-shaped int array of sub-row ids -> [..., 8] bf16-as-uint16 one-hot."""
    import ml_dtypes
    oh = (m[..., None] == np.arange(PACK)).astype(ml_dtypes.bfloat16)
    return oh


def prepare_inputs(inputs, P: Params):
    import ml_dtypes
    x = np.asarray(inputs["x"]); ei = np.asarray(inputs["edge_index"])
    M12, k12 = fuse_weights(
        inputs["W_e1"], inputs["b_e1"], inputs["W_e2"], inputs["b_e2"],
        inputs["W_lin"], inputs["b_lin"], inputs["W_att"],
        inputs["att_src"], inputs["att_dst"])
    src = ei[0].astype(np.int64); dst = ei[1].astype(np.int64)
    NS, NB, NC = P.NS, P.NB, P.NC
    core_of = dst // NS

    # shared degree-sort permutation per core + shared per-block col budget
    perms = []
    rankof_full = np.empty(P.N, np.int64)
    Dt = np.zeros(NB, np.int64)
    for c in range(NC):
        m = core_of == c
        dloc = dst[m] - c * NS
        deg = np.bincount(dloc, minlength=NS)
        order = np.argsort(-deg, kind="stable")
        perms.append(order)
        rank_of = np.empty(NS, np.int64)
        rank_of[order] = np.arange(NS)
        rankof_full[c * NS:(c + 1) * NS] = rank_of
        degp = np.zeros(NB * 128, np.int64)
        degp[:NS] = deg[order]
        Dt = np.maximum(Dt, degp.reshape(NB, 128).max(1))
    P.Dt = tuple(int(v) for v in Dt)
    # table rows are rank-ordered per core: gid of node n = core*(NS+1)+rank(n)
    gid_src = (src // NS) * (NS + 1) + rankof_full[src]
    Sq = P.Sq
    coff = np.zeros(NB, np.int64)
    coff[1:] = np.cumsum(Dt)[:-1]

    b_e1c = inputs["b_e1"].reshape(-1, 1).astype(np.float32)
    bias3 = np.zeros((128, 4), np.float32)
    bias3[:, :3] = inputs["bias"]

    in_maps = []
    for c in range(NC):
        m = core_of == c
        src_c = src[m]; dloc_c = dst[m] - c * NS
        gid_c = gid_src[m]
        perm = perms[c]
        rank_of = np.empty(NS, np.int64)
        rank_of[perm] = np.arange(NS)

        gid_dummy = c * (NS + 1) + NS
        # flat slot arrays (col-major within block: slot = (coff[b]+k)*128+p)
        LQ = np.full(128 * Sq, gid_dummy // PACK, np.int64)
        MQ = np.full(128 * Sq, gid_dummy % PACK, np.int64)

        r = rank_of[dloc_c]
        order_e = np.argsort(r, kind="stable")
        rs = r[order_e]; gs = gid_c[order_e]
        starts = np.searchsorted(rs, np.arange(NB * 128), side="left")
        k = np.arange(len(rs)) - starts[rs]
        p = rs % 128; b = rs // 128
        pos = (coff[b] + k) * 128 + p
        LQ[pos] = gs // PACK
        MQ[pos] = gs % PACK
        assert LQ.max() < P.NQ and LQ.min() >= 0

        # one-hot masks pre-expanded to the 12 payload lanes:
        # [128 p, Sq, PACK, 12] so the select multiply is fully packed bf16
        MQ_grid = MQ.reshape(Sq, 128).T           # [128, Sq]
        oh = onehot8_bf16(MQ_grid)                # [128, Sq, PACK]
        OM = np.ascontiguousarray(
            np.broadcast_to(oh[..., None], (128, Sq, PACK, 12))
        ).reshape(128, Sq * PACK * 12)

        # dense phase processes nodes in rank order (table rows = ranks)
        xs = np.zeros((P.NIN, NB * 128), ml_dtypes.bfloat16)
        xs[:, :NS] = x[c * NS:(c + 1) * NS][perm].T.astype(ml_dtypes.bfloat16)
        in_maps.append({
            "xT": xs, "W_e1": np.asarray(inputs["W_e1"]), "M12": M12, "k12": k12,
            "b_e1": b_e1c, "bias3": bias3,
            "SG": wrap_idx16(LQ), "OM": OM,
        })

    def post(results):
        outf = np.zeros((P.N, 3), np.float32)
        for c in range(NC):
            o = results[c]["out"].reshape(128, NB, 3)
            grid = np.transpose(o, (1, 0, 2)).reshape(NB * 128, 3)
            outf[c * NS + perms[c]] = grid[:NS]
        return outf

    return in_maps, post, perms


# ====================== entry point ======================

_CACHE = {}
last_exec_time_ns = None
last_result = None


def kernel(**inputs) -> np.ndarray:
    global last_exec_time_ns, last_result
    P = Params(N=100000, NC=8, NIN=768, EH=512)
    in_maps, post, _perms = prepare_inputs(inputs, P)

    key = ("gat8v2", P.N, P.Dt)
    if key not in _CACHE:
        nc = bacc.Bacc("TRN2", target_bir_lowering=False, debug=False,
                       num_devices=P.NC, num_swdge_queues=4)
        ins_ap = {}
        for name, arr in in_maps[0].items():
            ins_ap[name] = nc.dram_tensor(
                name, list(arr.shape), mybir.dt.from_np(arr.dtype),
                kind="ExternalInput").ap()
        out_ap = {"out": nc.dram_tensor(
            "out", [128, P.NB * 3], FP, kind="ExternalOutput").ap()}
        with tile.TileContext(nc) as tc:
            build_kernel(tc, out_ap, ins_ap, P)
        nc.compile()
        _CACHE[key] = nc
    nc = _CACHE[key]

    trace = os.environ.get("GAT_TRACE", "0") == "1"
    tmpdir = os.environ.get("GAT_TRACE_DIR") or None
    res = bass_utils.run_bass_kernel_spmd(
        nc, in_maps, core_ids=list(range(P.NC)), trace=trace, tmpdir=tmpdir)
    last_exec_time_ns = res.exec_time_ns
    last_result = res
    return post(res.results)

